# revision 1
# baseline (speedup 1.0000x reference)
"""LocalGaussianBlur (K=11, per-pixel sigma) Trainium2 Bass kernel.

Math: for each output pixel p=(h,w), with sigma = modulator[h,w]:
    var = 2*sigma^2 + 1e-8,  u = 1/var,  q = exp(-u)
    1-D kernel weights: e_t = exp(-t^2 * u) = q^(t^2), t = -5..5
    out[c,h,w] = (sum_{j,t} q^(j^2+t^2) * X[c,h+j,w+t]) / s^2,
    s = 1 + 2*(q + q^4 + q^9 + q^16 + q^25)

Since sigma in (0,1), q <= exp(-0.5) ~= 0.6065.  Terms with exponent
m = j^2 + t^2 > 31 contribute < ~1e-6 relative and are dropped.
Kept exponents (16): {0,1,2,4,5,8,9,10,13,16,17,18,20,25,26,29}.

Per core (8-way H-shard, 64 rows each + 5-row halo):
  layout [P = 96 partitions = 3 channels x 32 col-blocks of 16 cols,
          free dim = (rows, cols)]
  X tile [96, 74, 26] (row+col halos), weights computed redundantly for
  all 3 channel groups (modulator DMA'd 3x), so every elementwise op is
  a plain same-partition op with shifts expressed as free-dim offsets:
    A_t  = X[., w-t] + X[., w+t]                (col pair sums, t=1..5)
    C_jt = A_t[h-j, .] + A_t[h+j, .]            (row pair sums, j=1..5)
    Cm   = sum of C_jt/A_t-center with j^2+t^2 = m
    acc  = X_center + sum_m exp(-m*u) * Cm      (ACT makes the exp maps)
    out  = acc / s^2
"""

import os
import numpy as np

K = 11
PAD = 5
H = W = 512
C = 3
NCORES = 8
RS = H // NCORES          # 64 output rows per core
RH = RS + 2 * PAD         # 74 input rows per core
WB = 32                   # col blocks per partition-group
WBC = W // WB             # 16 cols per block
WHC = WBC + 2 * PAD       # 26 cols incl halo
P = C * WB                # 96 partitions
XCOLS = 536               # padded dram cols: 5 + 512 + 19

# exponent m -> list of (j, t) with j,t >= 1 (4-tap row+col pair groups)
# plus marker entries (0, t) handled via A_t center rows.
KEPT_M = [1, 2, 4, 5, 8, 9, 10, 13, 16, 17, 18, 20, 25, 26, 29]


def _pairs_for_m(m):
    """(j,t) with j>=1, t>=0, j^2+t^2 == m; and t0 if m is a square t^2."""
    pjs = []
    for j in range(1, 6):
        for t in range(0, 6):
            if j * j + t * t == m:
                pjs.append((j, t))
    t0 = None
    for t in range(1, 6):
        if t * t == m:
            t0 = t
    return pjs, t0


_NC_CACHE = {}


def _build_nc():
    if "nc" in _NC_CACHE:
        return _NC_CACHE["nc"]
    import concourse.bass as bass  # noqa: F401
    from concourse import bacc
    import concourse.mybir as mybir
    from concourse.tile import TileContext

    f32 = mybir.dt.float32
    bf16 = mybir.dt.bfloat16
    bf_mode = os.environ.get("LGB_BF16", "0")
    use_bf16 = bf_mode in ("1", "2")
    dmid = bf16 if use_bf16 else f32
    dacc = f32 if bf_mode == "2" else dmid
    AF = mybir.ActivationFunctionType
    ALU = mybir.AluOpType

    nc = bacc.Bacc()
    # staged in exact SBUF tile layout host-side (one DMA each, one writer
    # per tile: walrus caps per-instruction sync waits)
    x = nc.dram_tensor("x", [P, RH, WHC], dmid, kind="ExternalInput")
    md = nc.dram_tensor("md", [P, RS, WBC], f32, kind="ExternalInput")
    out = nc.dram_tensor("out", [C, RS, W], f32, kind="ExternalOutput")

    with TileContext(nc) as tc:
        nrep = int(os.environ.get("LGB_REPEAT", "1"))
        with (
            tc.tile_pool(name="big", bufs=1) as big,
            tc.tile_pool(name="cpool", bufs=int(os.environ.get("LGB_CBUFS", "8"))) as cpool,
            tc.tile_pool(name="qpool", bufs=int(os.environ.get("LGB_QBUFS", "3"))) as qpool,
        ):
            X = big.tile([P, RH, WHC], dmid, tag="X")
            MD = big.tile([P, RS, WBC], f32, tag="MD")

            # ---- input DMAs (host staged layout: one DMA per tile) ----
            nc.sync.dma_start(out=X[:], in_=x[:])
            nc.sync.dma_start(out=MD[:], in_=md[:])

            def body(emit_out):
                # ---- per-pixel u = 1/(2*sigma^2 + 1e-8) ----
                Vt = big.tile([P, RS, WBC], f32, tag="Vt", name="Vt")
                U = big.tile([P, RS, WBC], f32, tag="U", name="U")
                nc.scalar.activation(Vt[:], MD[:], AF.Square,
                                     scale=float(np.sqrt(2.0)))
                nc.vector.tensor_scalar_add(Vt[:], Vt[:], 1e-8)
                nc.vector.reciprocal(U[:], Vt[:])

                # ---- normalization 1/s^2 computed EARLY so the serial
                # chain (4 adds + scale + recip + square) overlaps the
                # combine instead of extending the kernel tail ----
                NRM = big.tile([P, RS, WBC], f32, tag="NRM", name="NRM")
                SQ = big.tile([P, RS, WBC], dmid, tag="SQ", name="SQ")
                qn_prev = None
                for i, mm in enumerate((1, 4, 9, 16, 25)):
                    qn = qpool.tile([P, RS, WBC], f32, tag="Qn", name="qn",
                                    bufs=2)
                    nc.scalar.activation(qn[:], U[:], AF.Exp, scale=float(-mm))
                    if i == 1:
                        nc.gpsimd.tensor_tensor(SQ[:], qn_prev[:], qn[:],
                                                ALU.add)
                    elif i > 1:
                        nc.gpsimd.tensor_tensor(SQ[:], SQ[:], qn[:], ALU.add)
                    qn_prev = qn
                nc.scalar.activation(NRM[:], SQ[:], AF.Copy, bias=1.0,
                                     scale=2.0)          # s = 2*sum + 1
                nc.vector.reciprocal(NRM[:], NRM[:])      # 1/s
                nc.scalar.activation(NRM[:], NRM[:], AF.Square)  # 1/s^2

                # ---- col pair sums A_t ----
                A = {}
                for t in range(1, 6):
                    A[t] = big.tile([P, RH, WBC], dmid, tag=f"A{t}",
                                    name=f"A{t}")
                    nc.vector.tensor_tensor(
                        A[t][:],
                        X[:, :, PAD - t : PAD - t + WBC],
                        X[:, :, PAD + t : PAD + t + WBC],
                        ALU.add,
                    )

                def a_center(t):
                    if t == 0:
                        return X[:, PAD : PAD + RS, PAD : PAD + WBC]
                    return A[t][:, PAD : PAD + RS, :]

                def a_rows(t, j):
                    if t == 0:
                        return (
                            X[:, PAD - j : PAD - j + RS, PAD : PAD + WBC],
                            X[:, PAD + j : PAD + j + RS, PAD : PAD + WBC],
                        )
                    return (
                        A[t][:, PAD - j : PAD - j + RS, :],
                        A[t][:, PAD + j : PAD + j + RS, :],
                    )

                ACC = big.tile([P, RS, WBC], dacc, tag="ACC", name="ACC")
                ACC2 = big.tile([P, RS, WBC], dacc, tag="ACC2", name="ACC2")
                TMP = big.tile([P, RS, WBC], dacc, tag="TMP", name="TMP")
                TMP2 = big.tile([P, RS, WBC], dacc, tag="TMP2", name="TMP2")
                # Each exponent-group runs wholly on ONE engine (DVE or
                # GPSIMD), each with its own accumulator; greedy balance
                # by modeled cost.
                C_DVE = 1.222
                # real-HW: GPSIMD fp32 TT ~3.8us/op (vs model 2.2) -- a
                # moderate offload (~12 ops) still beats all-DVE slightly
                C_GP = float(os.environ.get("LGB_C_GP", "6.5"))
                # recips + tsp + 5 A-ops + merge/final pre-booked on DVE
                eng_busy = {"dve": 2.9 + 5 * 1.4 + 3 * C_DVE, "gp": 0.0}
                ENG = {"dve": nc.vector, "gp": nc.gpsimd}
                accs = {}
                tmps = {"dve": TMP, "gp": TMP2}

                def pick(nops):
                    if (eng_busy["dve"] + nops * C_DVE
                            <= eng_busy["gp"] + nops * C_GP):
                        eng_busy["dve"] += nops * C_DVE
                        return "dve"
                    eng_busy["gp"] += nops * C_GP
                    return "gp"

                for m in KEPT_M:
                    pjs, t0 = _pairs_for_m(m)
                    nops = len(pjs) + (1 if t0 is not None else 0) \
                        + max(0, len(pjs) - 1) + 2
                    e = pick(nops)
                    eng = ENG[e]
                    parts = []
                    for (j, t) in pjs:
                        ct = cpool.tile([P, RS, WBC], dmid, tag="C",
                                        name="Cjt")
                        lo, hi = a_rows(t, j)
                        eng.tensor_tensor(ct[:], lo, hi, ALU.add)
                        parts.append(ct)
                    if t0 is not None:
                        eng.tensor_tensor(parts[0][:], parts[0][:],
                                          a_center(t0), ALU.add)
                    while len(parts) > 1:
                        eng.tensor_tensor(parts[0][:], parts[0][:],
                                          parts[1][:], ALU.add)
                        parts.pop(1)
                    cm = parts[0]

                    # weight map q^m = exp(-m*u)
                    qm = qpool.tile([P, RS, WBC], dmid, tag="Q", name="Qm")
                    nc.scalar.activation(qm[:], U[:], AF.Exp, scale=float(-m))

                    if e not in accs:
                        acc_t = ACC if e == "dve" else ACC2
                        eng.tensor_tensor(acc_t[:], qm[:], cm[:], ALU.mult)
                        accs[e] = acc_t
                    else:
                        eng.tensor_tensor(tmps[e][:], qm[:], cm[:], ALU.mult)
                        eng.tensor_tensor(accs[e][:], accs[e][:], tmps[e][:],
                                          ALU.add)

                # merge accumulators, + m = 0 term (X center)
                res = ACC if "dve" in accs else ACC2
                if "gp" in accs and "dve" in accs:
                    nc.vector.tensor_tensor(ACC[:], ACC[:], ACC2[:], ALU.add)
                nc.vector.tensor_tensor(res[:], res[:], a_center(0), ALU.add)

                if emit_out:
                    OUTT = big.tile([P, RS, WBC], f32, tag="OUTT",
                                    name="OUTT")
                    nc.vector.tensor_tensor(OUTT[:], res[:], NRM[:], ALU.mult)
                    for c in range(C):
                        nc.sync.dma_start(
                            out=out[c].rearrange("r (wb k) -> wb r k", k=WBC),
                            in_=OUTT[c * WB : (c + 1) * WB],
                        )
                else:
                    nc.vector.tensor_tensor(res[:], res[:], Vt[:], ALU.mult)


            # --- scan-Horner variant: per-pixel polynomial evaluated by
            # tensor_tensor_scan (state = q^gap * state + Cm), slots along
            # the innermost free dim, two 32-row halves for SBUF fit ---
            SLOTS = [29, 26, 25, 20, 18, 17, 16, 13, 10, 9, 8, 5, 4, 2, 1]
            NSLOT = len(SLOTS) + 1  # + m=0 (X center)
            GAPS = [0] + [SLOTS[i] - SLOTS[i + 1] for i in range(len(SLOTS) - 1)] + [1]

            def body_scan(emit_out):
                Vt = big.tile([P, RS, WBC], f32, tag="Vt", name="Vt")
                U = big.tile([P, RS, WBC], f32, tag="U", name="U")
                nc.scalar.activation(Vt[:], MD[:], AF.Square,
                                     scale=float(np.sqrt(2.0)))
                nc.vector.tensor_scalar_add(Vt[:], Vt[:], 1e-8)
                nc.vector.reciprocal(U[:], Vt[:])

                A = {}
                for t in range(1, 6):
                    A[t] = big.tile([P, RH, WBC], f32, tag=f"A{t}",
                                    name=f"A{t}")
                    nc.vector.tensor_tensor(
                        A[t][:],
                        X[:, :, PAD - t : PAD - t + WBC],
                        X[:, :, PAD + t : PAD + t + WBC],
                        ALU.add,
                    )

                HR = 32  # rows per half
                HPX = HR * WBC  # 512

                def flat(ap):
                    return ap.rearrange("p a b -> p (a b)")

                OUTT = big.tile([P, RS, WBC], f32, tag="OUTT", name="OUTT")

                for half in range(2):
                    r0 = half * HR
                    CC0 = big.tile([P, HPX, NSLOT], f32, tag="CC0", name="CC0")
                    CC1 = big.tile([P, HPX, NSLOT], f32, tag="CC1", name="CC1")
                    SCO = big.tile([P, HPX, NSLOT], f32, tag="SCO", name="SCO")
                    # row/col-shaped views of the slot tensors
                    CC0r = CC0.rearrange("p (a b) s -> p a b s", b=WBC)
                    CC1r = CC1.rearrange("p (a b) s -> p a b s", b=WBC)
                    SCOr = SCO.rearrange("p (a b) s -> p a b s", b=WBC)

                    def a_rows_h(t, j):
                        lo = PAD + r0 - j
                        hi = PAD + r0 + j
                        if t == 0:
                            return (
                                X[:, lo : lo + HR, PAD : PAD + WBC],
                                X[:, hi : hi + HR, PAD : PAD + WBC],
                            )
                        return (
                            A[t][:, lo : lo + HR, :],
                            A[t][:, hi : hi + HR, :],
                        )

                    def a_center_h(t):
                        if t == 0:
                            return X[:, PAD + r0 : PAD + r0 + HR,
                                     PAD : PAD + WBC]
                        return A[t][:, PAD + r0 : PAD + r0 + HR, :]

                    nc.vector.memset(CC0r[:, :, :, 0], 0.0)
                    uh = U[:, r0 : r0 + HR, :]
                    for s, m in enumerate(SLOTS):
                        slot1 = CC1r[:, :, :, s]
                        pjs, t0 = _pairs_for_m(m)
                        parts = []
                        for (j, t) in pjs:
                            lo, hi = a_rows_h(t, j)
                            if len(pjs) == 1 and t0 is None:
                                nc.vector.tensor_tensor(slot1, lo, hi, ALU.add)
                                parts = None
                                break
                            ct = cpool.tile([P, HR, WBC], f32, tag="C",
                                            name="Cjt")
                            nc.vector.tensor_tensor(ct[:], lo, hi, ALU.add)
                            parts.append(ct)
                        if parts is not None:
                            run = parts[0][:]
                            rest = []
                            if t0 is not None:
                                rest.append(a_center_h(t0))
                            rest.extend(pp[:] for pp in parts[1:])
                            for i, rr in enumerate(rest):
                                dst = slot1 if i == len(rest) - 1 else run
                                nc.vector.tensor_tensor(dst, run, rr, ALU.add)
                        if GAPS[s] > 0:
                            nc.scalar.activation(CC0r[:, :, :, s], uh, AF.Exp,
                                                 scale=float(-GAPS[s]))
                    # slot 15: m=0 -> X center, gap 1
                    nc.scalar.activation(CC1r[:, :, :, NSLOT - 1],
                                         a_center_h(0), AF.Copy)
                    nc.scalar.activation(CC0r[:, :, :, NSLOT - 1], uh, AF.Exp,
                                         scale=-1.0)

                    nc.vector.tensor_tensor_scan(
                        flat(SCO[:, :, :]), flat(CC0[:, :, :]),
                        flat(CC1[:, :, :]), 0.0, ALU.mult, ALU.add)

                    # stash result slice into OUTT rows (unnormalized)
                    nc.vector.tensor_copy(
                        OUTT[:, r0 : r0 + HR, :], SCOr[:, :, :, NSLOT - 1])

                # ---- normalization ----
                SQ = big.tile([P, RS, WBC], f32, tag="SQ", name="SQ")
                q1 = qpool.tile([P, RS, WBC], f32, tag="Q", name="q1")
                nc.scalar.activation(q1[:], U[:], AF.Exp, scale=-1.0)
                first = True
                for mm in (4, 9, 16, 25):
                    qq = qpool.tile([P, RS, WBC], f32, tag="Q", name="qq")
                    nc.scalar.activation(qq[:], U[:], AF.Exp, scale=float(-mm))
                    if first:
                        nc.vector.tensor_tensor(SQ[:], q1[:], qq[:], ALU.add)
                        first = False
                    else:
                        nc.vector.tensor_tensor(SQ[:], SQ[:], qq[:], ALU.add)
                nc.scalar.activation(Vt[:], SQ[:], AF.Copy, bias=1.0,
                                     scale=2.0)
                nc.vector.reciprocal(Vt[:], Vt[:])
                nc.scalar.activation(Vt[:], Vt[:], AF.Square)  # 1/s^2

                nc.vector.tensor_tensor(OUTT[:], OUTT[:], Vt[:], ALU.mult)
                if emit_out:
                    for c in range(C):
                        nc.sync.dma_start(
                            out=out[c].rearrange("r (wb k) -> wb r k", k=WBC),
                            in_=OUTT[c * WB : (c + 1) * WB],
                        )

            use_scan = os.environ.get("LGB_SCAN", "0") == "1"
            for rep in range(nrep):
                (body_scan if use_scan else body)(emit_out=(rep == nrep - 1))

    nc.compile()
    _NC_CACHE["nc"] = nc
    return nc


def _stage_inputs(img, modulator):
    """Host-side shard staging: replicate-pad, halo-duplicate into the
    exact SBUF tile layout [96, rows, cols] per core."""
    img = np.ascontiguousarray(np.asarray(img, dtype=np.float32))
    modulator = np.ascontiguousarray(np.asarray(modulator, dtype=np.float32))
    x = img[0]  # (3, 512, 512)
    xp = np.pad(x, ((0, 0), (PAD, PAD), (PAD, PAD)), mode="edge")  # (3,522,522)
    in_maps = []
    for i in range(NCORES):
        r0 = i * RS
        xs = xp[:, r0 : r0 + RH, :]  # (3, 74, 522)
        # partition p = c*WB + wb  ->  xt2[c*WB+wb] = xs[c,:,wb*16:wb*16+26]
        xdt = np.float32
        if os.environ.get("LGB_BF16", "0") in ("1", "2"):
            import ml_dtypes
            xdt = ml_dtypes.bfloat16
        xt2 = np.empty((P, RH, WHC), dtype=xdt)
        for c in range(C):
            for wb in range(WB):
                xt2[c * WB + wb] = xs[c, :, wb * WBC : wb * WBC + WHC]
        mds = modulator[r0 : r0 + RS, :]  # (64, 512)
        mdt = np.empty((P, RS, WBC), dtype=np.float32)
        for c in range(C):
            for wb in range(WB):
                mdt[c * WB + wb] = mds[:, wb * WBC : (wb + 1) * WBC]
        in_maps.append(
            {"x": np.ascontiguousarray(xt2), "md": np.ascontiguousarray(mdt)}
        )
    return in_maps


def kernel(img, modulator):
    from concourse.bass_utils import run_bass_kernel_spmd

    nc = _build_nc()
    in_maps = _stage_inputs(img, modulator)
    res = run_bass_kernel_spmd(nc, in_maps, list(range(NCORES))).results
    out = np.concatenate(
        [np.asarray(res[i]["out"]).reshape(C, RS, W) for i in range(NCORES)],
        axis=1,
    )
    return np.ascontiguousarray(out[None], dtype=np.float32)  # (1,3,512,512)



# revision 2
# speedup vs baseline: 5.3398x; 5.3398x over previous
"""LocalGaussianBlur v2 — Trainium2 Bass kernel (7x7 truncation).

Math: sigma = modulator[h,w] in (0,1); u = 1/(2 sigma^2 + 1e-8);
q = exp(-u) <= exp(-0.5).  Weight of tap (j,t) is q^(j^2+t^2).
Since q <= 0.6065, taps with |j| or |t| >= 4 carry < 5.4e-4 of the
kernel mass; normalizing by the truncated sum s3 = 1 + 2(q+q^4+q^9)
makes the truncated kernel a proper weighted average, so rel err from
truncation is ~5e-4.  m = j^2+t^2 groups kept: {1,2,4,5,8,9,10} and
m=10's partner merged as q^10 C10 ~= q^9 * (0.45 C10) (worst ~0.22%).

out = [Xc + q C1 + q^2 C2 + q^4 C4 + q^5 C5 + q^8 C8
        + q^9 (C9 + 0.45 C10)] / s3^2

Layout per core (8-way H-shard, 64 rows each):
  partitions p = rq*32+cb (4 row-quarters x 32 col-blocks),
  per-partition spatial block 16x16, X with halo rows 3 / cols 4
  -> X tile [128, 3ch, 22, 24] bf16.  A second copy XO shifted by one
  column keeps every column-pair add 4B-aligned for the DVE 2x bf16
  mode.  Weight maps are per-pixel [128,16,16], broadcast over the
  channel axis with stride-0 APs.
"""

import os
import numpy as np

H = W = 512
C = 3
NC = 8
RS = H // NC        # 64 rows per core
RQ = 4              # row-quarters per core
CB = 32             # col blocks
TR = 16             # block rows
TC = 16             # block cols
RHL = 3             # row halo
CHL = 4             # col halo (even => aligned bf16 slices)
XR = TR + 2 * RHL   # 22
XC = TC + 2 * CHL   # 24
P = 128

_NC_CACHE = {}


def _build_nc():
    if "nc" in _NC_CACHE:
        return _NC_CACHE["nc"]
    import concourse.bass as bass  # noqa: F401
    from concourse import bacc
    import concourse.mybir as mybir
    from concourse.tile import TileContext
    from concourse.ap import AP as BassAP

    f32 = mybir.dt.float32
    bf16 = mybir.dt.bfloat16
    AF = mybir.ActivationFunctionType
    ALU = mybir.AluOpType

    nc = bacc.Bacc()
    x = nc.dram_tensor("x", [P, C, XR, XC], bf16, kind="ExternalInput")
    xo = nc.dram_tensor("xo", [P, C, XR, XC], bf16, kind="ExternalInput")
    md = nc.dram_tensor("md", [P, TR, TC], f32, kind="ExternalInput")
    out = nc.dram_tensor("out", [P, C, TR, TC], f32, kind="ExternalOutput")

    use_gp = os.environ.get("LGB2_GP", "0") == "1"
    nrep = int(os.environ.get("LGB2_REPEAT", "1"))

    with TileContext(nc) as tc:
        with tc.tile_pool(name="main", bufs=1) as pool:
            X = pool.tile([P, C, XR, XC], bf16, tag="X")
            XO = pool.tile([P, C, XR, XC], bf16, tag="XO")
            MD = pool.tile([P, TR, TC], f32, tag="MD")
            nc.sync.dma_start(out=MD[:], in_=md[:])
            nc.sync.dma_start(out=X[:], in_=x[:])
            nc.sync.dma_start(out=XO[:], in_=xo[:])

            # all tiles allocated up-front so the compute body can sit
            # inside a hardware loop (repeat-timing mode)
            V = pool.tile([P, TR, TC], f32, tag="V")
            U = pool.tile([P, TR, TC], f32, tag="U")
            # slots: q1,q4,q9,q2,q5,q10
            QAB = pool.tile([P, 6, TR, TC], bf16, tag="QAB")
            Q8 = pool.tile([P, TR, TC], bf16, tag="Q8")
            SS = pool.tile([P, TR, TC], f32, tag="SS")
            RN = pool.tile([P, TR, TC], f32, tag="RN")
            NRM = pool.tile([P, TR, TC], f32, tag="NRM")
            A = pool.tile([P, 3, C, XR, TC], bf16, tag="A")
            # CC slots: C1, C4, C9, C2, C5, C10
            CC = pool.tile([P, 6, C, TR, TC], bf16, tag="CC")
            J23 = pool.tile([P, 3, C, TR, TC], bf16, tag="J23")  # C5b,C8,C10b
            # PR slots: P1, P4, P9, P2, P5, P10, P8
            PR = pool.tile([P, 7, C, TR, TC], bf16, tag="PR")
            L1 = pool.tile([P, 3, C, TR, TC], bf16, tag="L1")
            W1 = pool.tile([P, C, TR, TC], bf16, tag="W1")
            W2 = pool.tile([P, C, TR, TC], bf16, tag="W2")
            F1 = pool.tile([P, C, TR, TC], f32, tag="F1")
            F2 = pool.tile([P, C, TR, TC], f32, tag="F2")
            OUTT = pool.tile([P, C, TR, TC], f32, tag="OUTT")

            def body():
                # ---- per-pixel u = 1/(2 sigma^2 + 1e-8) (ACT+DVE) ----
                nc.scalar.activation(V[:], MD[:], AF.Square,
                                     scale=float(np.sqrt(2.0)))
                nc.vector.tensor_scalar_add(V[:], V[:], 1e-8)
                nc.vector.reciprocal_approx_fast(U[:], V[:])

                # ---- weight maps on ACT (overlap the DVE pair-sum stage) ----
                for i, m in enumerate((1, 4, 9, 2, 5, 10)):
                    nc.scalar.activation(QAB[:, i], U[:], AF.Exp,
                                         scale=float(-m))
                nc.scalar.activation(Q8[:], U[:], AF.Exp, scale=-8.0)

                # ---- A_t: column pair sums (t-slot, ch, rows incl halo);
                # independent of the ACT weight chain, so the DVE stays busy
                # while the exps cook ----
                nc.vector.tensor_tensor(
                    A[:, 0], XO[:, :, :, 2:2 + TC], XO[:, :, :, 4:4 + TC],
                    ALU.add)
                nc.vector.tensor_tensor(
                    A[:, 1], X[:, :, :, 2:2 + TC], X[:, :, :, 6:6 + TC],
                    ALU.add)
                nc.vector.tensor_tensor(
                    A[:, 2], XO[:, :, :, 0:TC], XO[:, :, :, 6:6 + TC],
                    ALU.add)

                # ---- normalization 1/s^2, s = 1+2(q1+q4+q9): early, so the
                # ACT ops (scale, square) finish well before the output
                # multiply needs NRM ----
                nc.vector.tensor_tensor(SS[:], QAB[:, 0], QAB[:, 1], ALU.add)
                nc.vector.tensor_tensor(SS[:], SS[:], QAB[:, 2], ALU.add)
                nc.scalar.activation(RN[:], SS[:], AF.Copy, bias=1.0,
                                     scale=2.0)
                nc.vector.reciprocal_approx_fast(RN[:], RN[:])
                nc.scalar.activation(NRM[:], RN[:], AF.Square)

                # ---- X row-pairs (j=1,2,3) in ONE op: the slot axis walks
                # the row offset (stride -XC / +XC elements) -> CC[0:3];
                # then += A center rows -> CC slots (C1, C4, C9) ----
                def xslot(j0, slot_stride):
                    b = X[:, None, :, j0:j0 + TR, CHL:CHL + TC] \
                        .broadcast_to([P, 3, C, TR, TC])
                    ap2 = list(b.ap)
                    ap2[1] = [slot_stride * XC, 3]
                    return BassAP(b.tensor, b.offset, ap2)

                if os.environ.get("LGB2_XMERGE", "0") == "1":
                    nc.vector.tensor_tensor(
                        CC[:, 0:3], xslot(RHL - 1, -1), xslot(RHL + 1, 1),
                        ALU.add)
                else:
                    for i, j in enumerate((1, 2, 3)):
                        nc.vector.tensor_tensor(
                            CC[:, i],
                            X[:, :, RHL - j:RHL - j + TR, CHL:CHL + TC],
                            X[:, :, RHL + j:RHL + j + TR, CHL:CHL + TC],
                            ALU.add)
                nc.vector.tensor_tensor(
                    CC[:, 0:3], CC[:, 0:3], A[:, :, :, RHL:RHL + TR, :],
                    ALU.add)

                def arows(sl, j):
                    return (A[:, sl, :, RHL - j:RHL - j + TR, :],
                            A[:, sl, :, RHL + j:RHL + j + TR, :])

                # ---- A row-pairs ----
                lo, hi = arows(slice(0, 3), 1)
                nc.vector.tensor_tensor(CC[:, 3:6], lo, hi, ALU.add)
                # -> CC (.., C2, C5a, C10a)
                lo, hi = arows(slice(0, 2), 2)
                nc.vector.tensor_tensor(J23[:, 0:2], lo, hi, ALU.add)  # C5b,C8
                lo, hi = arows(0, 3)
                nc.vector.tensor_tensor(J23[:, 2], lo, hi, ALU.add)    # C10b
                # C5 += C5b ; C10 = C10a + C10b   (strided slot pair, one op)
                j13 = J23[:, 0:3:2]
                nc.vector.tensor_tensor(CC[:, 4:6], CC[:, 4:6], j13, ALU.add)

                # ---- products ----
                def bc3(q):  # [P,TR,TC] -> [P,C,TR,TC] stride-0 channel bc
                    return q[:, None, :, :].broadcast_to([P, C, TR, TC])

                nc.vector.tensor_tensor(
                    PR[:, 0:6],
                    QAB[:, :, None, :, :].broadcast_to([P, 6, C, TR, TC]),
                    CC[:], ALU.mult)    # (P1, P4, P9, P2, P5, P10)
                nc.vector.tensor_tensor(PR[:, 6], bc3(Q8), J23[:, 1],
                                        ALU.mult)  # P8

                # ---- reduction tree (small terms in bf16, tail in fp32) ----
                nc.vector.tensor_tensor(L1[:], PR[:, 1:4], PR[:, 4:7],
                                        ALU.add)   # (P4+P2, P9+P5, P10+P8)
                nc.vector.tensor_tensor(W1[:], L1[:, 1], L1[:, 2], ALU.add)
                nc.vector.tensor_tensor(W2[:], W1[:], L1[:, 0], ALU.add)
                nc.vector.tensor_tensor(F1[:], W2[:], PR[:, 0], ALU.add)
                nc.vector.tensor_tensor(
                    F2[:], F1[:], X[:, :, RHL:RHL + TR, CHL:CHL + TC], ALU.add)
                nc.vector.tensor_tensor(OUTT[:], F2[:], bc3(NRM), ALU.mult)

            if nrep == 1:
                body()
            else:
                with tc.For_i(0, nrep, 1):
                    body()
            nc.sync.dma_start(out=out[:], in_=OUTT[:])

    nc.compile()
    _NC_CACHE["nc"] = nc
    return nc


def _stage_inputs(img, modulator):
    import ml_dtypes
    x = np.ascontiguousarray(np.asarray(img, dtype=np.float32))[0]  # (3,H,W)
    mod = np.ascontiguousarray(np.asarray(modulator, dtype=np.float32))
    xpad = np.pad(x, ((0, 0), (RHL, RHL), (CHL, CHL + 1)), mode="edge")
    # (3, 518, 521)
    idx_r = (np.arange(RQ) * TR)[:, None] + np.arange(XR)[None, :]  # (4,22)
    idx_c = (np.arange(CB) * TC)[:, None] + np.arange(XC)[None, :]  # (32,24)
    mir = (np.arange(RQ) * TR)[:, None] + np.arange(TR)[None, :]
    mic = (np.arange(CB) * TC)[:, None] + np.arange(TC)[None, :]
    in_maps = []
    for core in range(NC):
        sub = xpad[:, core * RS:core * RS + RS + 2 * RHL, :]  # (3,70,521)
        # (3, 4, 32, 22, 24) -> (128, 3, 22, 24)
        blk = sub[:, idx_r[:, None, :, None], idx_c[None, :, None, :]]
        xt = np.ascontiguousarray(
            blk.transpose(1, 2, 0, 3, 4).reshape(P, C, XR, XC)
        ).astype(ml_dtypes.bfloat16)
        blk_o = sub[:, idx_r[:, None, :, None], idx_c[None, :, None, :] + 1]
        xot = np.ascontiguousarray(
            blk_o.transpose(1, 2, 0, 3, 4).reshape(P, C, XR, XC)
        ).astype(ml_dtypes.bfloat16)
        msub = mod[core * RS:core * RS + RS, :]  # (64, 512)
        mdt = np.ascontiguousarray(
            msub[mir[:, None, :, None], mic[None, :, None, :]]
            .reshape(P, TR, TC))
        in_maps.append({"x": xt, "xo": xot, "md": mdt})
    return in_maps


def kernel(img, modulator):
    from concourse.bass_utils import run_bass_kernel_spmd

    nc = _build_nc()
    in_maps = _stage_inputs(img, modulator)
    res = run_bass_kernel_spmd(nc, in_maps, list(range(NC))).results
    # per-core out [128, 3, 16, 16] -> (3, 64, 512)
    parts = []
    for i in range(NC):
        o = np.asarray(res[i]["out"]).reshape(RQ, CB, C, TR, TC)
        parts.append(o.transpose(2, 0, 3, 1, 4).reshape(C, RS, W))
    out = np.concatenate(parts, axis=1)
    return np.ascontiguousarray(out[None], dtype=np.float32)


# revision 3
# speedup vs baseline: 6.3037x; 1.1805x over previous
"""LocalGaussianBlur v2 — Trainium2 Bass kernel (7x7 truncation).

Math: sigma = modulator[h,w] in (0,1); u = 1/(2 sigma^2 + 1e-8);
q = exp(-u) <= exp(-0.5).  Weight of tap (j,t) is q^(j^2+t^2).
Since q <= 0.6065, taps with |j| or |t| >= 4 carry < 5.4e-4 of the
kernel mass; normalizing by the truncated sum s3 = 1 + 2(q+q^4+q^9)
makes the truncated kernel a proper weighted average, so rel err from
truncation is ~5e-4.  m = j^2+t^2 groups kept: {1,2,4,5,8,9,10} and
m=10's partner merged as q^10 C10 ~= q^9 * (0.45 C10) (worst ~0.22%).

out = [Xc + q C1 + q^2 C2 + q^4 C4 + q^5 C5 + q^8 C8
        + q^9 (C9 + 0.45 C10)] / s3^2

Layout per core (8-way H-shard, 64 rows each):
  partitions p = rq*32+cb (4 row-quarters x 32 col-blocks),
  per-partition spatial block 16x16, X with halo rows 3 / cols 4
  -> X tile [128, 3ch, 22, 24] bf16.  A second copy XO shifted by one
  column keeps every column-pair add 4B-aligned for the DVE 2x bf16
  mode.  Weight maps are per-pixel [128,16,16], broadcast over the
  channel axis with stride-0 APs.
"""

import os
import numpy as np

H = W = 512
C = 3
NC = 8
RS = H // NC        # 64 rows per core
RQ = 4              # row-quarters per core
CB = 32             # col blocks
TR = 16             # block rows
TC = 16             # block cols
RHL = 3             # row halo
CHL = 4             # col halo (even => aligned bf16 slices)
XR = TR + 2 * RHL   # 22
XC = TC + 2 * CHL   # 24
P = 128

_NC_CACHE = {}


def _build_nc():
    if "nc" in _NC_CACHE:
        return _NC_CACHE["nc"]
    import concourse.bass as bass  # noqa: F401
    from concourse import bacc
    import concourse.mybir as mybir
    from concourse.tile import TileContext
    from concourse.ap import AP as BassAP

    f32 = mybir.dt.float32
    bf16 = mybir.dt.bfloat16
    AF = mybir.ActivationFunctionType
    ALU = mybir.AluOpType

    nc = bacc.Bacc()
    x = nc.dram_tensor("x", [P, C, XR, XC], bf16, kind="ExternalInput")
    xo = nc.dram_tensor("xo", [P, C, XR, XC], bf16, kind="ExternalInput")
    xn = nc.dram_tensor("xn", [P, C, XR, TC], bf16, kind="ExternalInput")
    md = nc.dram_tensor("md", [P, TR, TC], f32, kind="ExternalInput")
    out = nc.dram_tensor("out", [P, C, TR, TC], f32, kind="ExternalOutput")

    use_gp = os.environ.get("LGB2_GP", "0") == "1"
    nrep = int(os.environ.get("LGB2_REPEAT", "1"))

    with TileContext(nc) as tc:
        with tc.tile_pool(name="main", bufs=1) as pool:
            X = pool.tile([P, C, XR, XC], bf16, tag="X")
            XO = pool.tile([P, C, XR, XC], bf16, tag="XO")
            # XN: center cols only (no col halo) -> rows are contiguous, so
            # slot-strided multi-row-pair APs collapse to 3 free dims
            XN = pool.tile([P, C, XR, TC], bf16, tag="XN")
            MD = pool.tile([P, TR, TC], f32, tag="MD")
            nc.sync.dma_start(out=MD[:], in_=md[:])
            nc.sync.dma_start(out=X[:], in_=x[:])
            nc.sync.dma_start(out=XO[:], in_=xo[:])
            nc.sync.dma_start(out=XN[:], in_=xn[:])
            EPS = pool.tile([P, 1], f32, tag="EPS")
            nc.vector.memset(EPS[:], 1e-4)

            # all tiles allocated up-front so the compute body can sit
            # inside a hardware loop (repeat-timing mode)
            V = pool.tile([P, TR, TC], f32, tag="V")
            U = pool.tile([P, TR, TC], f32, tag="U")
            # slots: q1,q4,q9,q2,q5,q10,q8
            QAB = pool.tile([P, 7, TR, TC], bf16, tag="QAB")
            SS = pool.tile([P, TR, TC], f32, tag="SS")
            RN = pool.tile([P, TR, TC], f32, tag="RN")
            NRM = pool.tile([P, TR, TC], f32, tag="NRM")
            A = pool.tile([P, 3, C, XR, TC], bf16, tag="A")
            # CC slots: C1, C4, C9, C2, C5, C10, C8 | scratch: C5b, C10b
            CC = pool.tile([P, 9, C, TR, TC], bf16, tag="CC")
            # PR slots: P1, P4, P9, P2, P5, P10, P8
            PR = pool.tile([P, 7, C, TR, TC], bf16, tag="PR")
            L1 = pool.tile([P, 3, C, TR, TC], bf16, tag="L1")
            W1 = pool.tile([P, C, TR, TC], bf16, tag="W1")
            W2 = pool.tile([P, C, TR, TC], bf16, tag="W2")
            F1 = pool.tile([P, C, TR, TC], f32, tag="F1")
            F2 = pool.tile([P, C, TR, TC], f32, tag="F2")
            OUTT = pool.tile([P, C, TR, TC], f32, tag="OUTT")

            def body():
                # ACT head: V = 2*sigma^2 + eps starts immediately
                nc.scalar.activation(V[:], MD[:], AF.Square,
                                     scale=float(np.sqrt(2.0)), bias=EPS[:])

                # ---- A_t: column pair sums (t-slot, ch, rows incl halo);
                # independent of the weight chain -> DVE is busy while ACT
                # squares and the u-chain result is awaited ----
                nc.vector.tensor_tensor(
                    A[:, 0], XO[:, :, :, 2:2 + TC], XO[:, :, :, 4:4 + TC],
                    ALU.add)
                nc.vector.tensor_tensor(
                    A[:, 1], X[:, :, :, 2:2 + TC], X[:, :, :, 6:6 + TC],
                    ALU.add)
                nc.vector.tensor_tensor(
                    A[:, 2], XO[:, :, :, 0:TC], XO[:, :, :, 6:6 + TC],
                    ALU.add)

                # ---- per-pixel u = 1/(2 sigma^2 + eps); eps folded into the
                # Square's bias ((r2*sig + 1e-4)^2 = 2 sig^2 + 1e-8 + tiny
                # cross term ~2.8e-4*sig, a ~3e-4 relative wobble on u) ----
                nc.vector.reciprocal_approx_fast(U[:], V[:])

                # ---- weight maps on ACT (overlap the DVE pair-sum stage) ----
                for i, m in enumerate((1, 4, 9, 2, 5, 10, 8)):
                    nc.scalar.activation(QAB[:, i], U[:], AF.Exp,
                                         scale=float(-m))

                # ---- X row-pairs (j=1,2,3) in ONE op: the slot axis walks
                # the row offset (stride -TC / +TC elements of the halo-free
                # XN copy, so dims collapse to 3) -> CC[0:3]; then += A
                # center rows -> CC slots (C1, C4, C9) ----
                def xslot(j0, slot_stride):
                    b = XN[:, None, :, j0:j0 + TR, :] \
                        .broadcast_to([P, 3, C, TR, TC])
                    ap2 = list(b.ap)
                    ap2[1] = [slot_stride * TC, 3]
                    return BassAP(b.tensor, b.offset, ap2)

                if os.environ.get("LGB2_XMERGE", "1") == "1":
                    nc.vector.tensor_tensor(
                        CC[:, 0:3], xslot(RHL - 1, -1), xslot(RHL + 1, 1),
                        ALU.add)
                else:
                    for i, j in enumerate((1, 2, 3)):
                        nc.vector.tensor_tensor(
                            CC[:, i],
                            XN[:, :, RHL - j:RHL - j + TR, :],
                            XN[:, :, RHL + j:RHL + j + TR, :],
                            ALU.add)
                nc.vector.tensor_tensor(
                    CC[:, 0:3], CC[:, 0:3], A[:, :, :, RHL:RHL + TR, :],
                    ALU.add)

                # ---- normalization 1/s^2 = exp(-2 ln s), s = 1+2(q1+q4+q9):
                # the recip moves to ACT (ln+exp share one table set) ----
                nc.vector.tensor_tensor(SS[:], QAB[:, 0], QAB[:, 1], ALU.add)
                nc.vector.tensor_tensor(SS[:], SS[:], QAB[:, 2], ALU.add)
                nc.scalar.activation(RN[:], SS[:], AF.Copy, bias=1.0,
                                     scale=2.0)
                nc.scalar.activation(RN[:], RN[:], AF.Ln)
                nc.scalar.activation(NRM[:], RN[:], AF.Exp, scale=-2.0)

                def arows(sl, j):
                    return (A[:, sl, :, RHL - j:RHL - j + TR, :],
                            A[:, sl, :, RHL + j:RHL + j + TR, :])

                # ---- A row-pairs ----
                lo, hi = arows(slice(0, 3), 1)
                nc.vector.tensor_tensor(CC[:, 3:6], lo, hi, ALU.add)
                # -> CC (.., C2, C5a, C10a)
                # j=2 pairs (C5b, C8) written slot-REVERSED so C8 lands at
                # CC[6] (joins the 7-slot product) and C5b at CC[7] (scratch)
                lo, hi = arows(slice(0, 2), 2)

                def rev2(apv):
                    ap2 = list(apv.ap)
                    sl = ap2[1]
                    assert sl[1] == 2
                    off = apv.offset + sl[0]
                    return BassAP(apv.tensor, off, [ap2[0], [-sl[0], 2]]
                                  + ap2[2:])

                nc.vector.tensor_tensor(rev2(CC[:, 6:8]), lo, hi, ALU.add)
                lo, hi = arows(0, 3)
                nc.vector.tensor_tensor(CC[:, 8], lo, hi, ALU.add)   # C10b
                # C5 += C5b ; C10 += C10b   (contiguous slot pair, one op)
                nc.vector.tensor_tensor(CC[:, 4:6], CC[:, 4:6], CC[:, 7:9],
                                        ALU.add)

                # ---- products ----
                def bc3(q):  # [P,TR,TC] -> [P,C,TR,TC] stride-0 channel bc
                    return q[:, None, :, :].broadcast_to([P, C, TR, TC])

                nc.vector.tensor_tensor(
                    PR[:],
                    QAB[:, :, None, :, :].broadcast_to([P, 7, C, TR, TC]),
                    CC[:, 0:7], ALU.mult)  # (P1, P4, P9, P2, P5, P10, P8)

                # ---- reduction tree (small terms in bf16, tail in fp32) ----
                nc.vector.tensor_tensor(L1[:], PR[:, 1:4], PR[:, 4:7],
                                        ALU.add)   # (P4+P2, P9+P5, P10+P8)
                nc.vector.tensor_tensor(W1[:], L1[:, 1], L1[:, 2], ALU.add)
                nc.vector.tensor_tensor(W2[:], W1[:], L1[:, 0], ALU.add)
                nc.vector.tensor_tensor(F1[:], W2[:], PR[:, 0], ALU.add)
                nc.vector.tensor_tensor(
                    F2[:], F1[:], XN[:, :, RHL:RHL + TR, :], ALU.add)
                nc.vector.tensor_tensor(OUTT[:], F2[:], bc3(NRM), ALU.mult)

            if nrep == 1:
                body()
            else:
                # 8x unrolled hw loop: the per-iteration For_i machinery
                # (~1.4 us) amortizes over 8 serial bodies in timing mode
                UN = 8
                assert nrep % UN == 0, nrep
                with tc.For_i(0, nrep // UN, 1):
                    for _ in range(UN):
                        body()
            nc.sync.dma_start(out=out[:], in_=OUTT[:])

    nc.compile()
    _NC_CACHE["nc"] = nc
    return nc


def _stage_inputs(img, modulator):
    import ml_dtypes
    x = np.ascontiguousarray(np.asarray(img, dtype=np.float32))[0]  # (3,H,W)
    mod = np.ascontiguousarray(np.asarray(modulator, dtype=np.float32))
    xpad = np.pad(x, ((0, 0), (RHL, RHL), (CHL, CHL + 1)), mode="edge")
    # (3, 518, 521)
    idx_r = (np.arange(RQ) * TR)[:, None] + np.arange(XR)[None, :]  # (4,22)
    idx_c = (np.arange(CB) * TC)[:, None] + np.arange(XC)[None, :]  # (32,24)
    idx_cn = (np.arange(CB) * TC)[:, None] + CHL + np.arange(TC)[None, :]
    mir = (np.arange(RQ) * TR)[:, None] + np.arange(TR)[None, :]
    mic = (np.arange(CB) * TC)[:, None] + np.arange(TC)[None, :]
    in_maps = []
    for core in range(NC):
        sub = xpad[:, core * RS:core * RS + RS + 2 * RHL, :]  # (3,70,521)
        # (3, 4, 32, 22, 24) -> (128, 3, 22, 24)
        blk = sub[:, idx_r[:, None, :, None], idx_c[None, :, None, :]]
        xt = np.ascontiguousarray(
            blk.transpose(1, 2, 0, 3, 4).reshape(P, C, XR, XC)
        ).astype(ml_dtypes.bfloat16)
        blk_o = sub[:, idx_r[:, None, :, None], idx_c[None, :, None, :] + 1]
        xot = np.ascontiguousarray(
            blk_o.transpose(1, 2, 0, 3, 4).reshape(P, C, XR, XC)
        ).astype(ml_dtypes.bfloat16)
        blk_n = sub[:, idx_r[:, None, :, None], idx_cn[None, :, None, :]]
        xnt = np.ascontiguousarray(
            blk_n.transpose(1, 2, 0, 3, 4).reshape(P, C, XR, TC)
        ).astype(ml_dtypes.bfloat16)
        msub = mod[core * RS:core * RS + RS, :]  # (64, 512)
        mdt = np.ascontiguousarray(
            msub[mir[:, None, :, None], mic[None, :, None, :]]
            .reshape(P, TR, TC))
        in_maps.append({"x": xt, "xo": xot, "xn": xnt, "md": mdt})
    return in_maps


def kernel(img, modulator):
    from concourse.bass_utils import run_bass_kernel_spmd

    nc = _build_nc()
    in_maps = _stage_inputs(img, modulator)
    res = run_bass_kernel_spmd(nc, in_maps, list(range(NC))).results
    # per-core out [128, 3, 16, 16] -> (3, 64, 512)
    parts = []
    for i in range(NC):
        o = np.asarray(res[i]["out"]).reshape(RQ, CB, C, TR, TC)
        parts.append(o.transpose(2, 0, 3, 1, 4).reshape(C, RS, W))
    out = np.concatenate(parts, axis=1)
    return np.ascontiguousarray(out[None], dtype=np.float32)


# revision 4
# speedup vs baseline: 6.6869x; 1.0608x over previous
"""LocalGaussianBlur v2 — Trainium2 Bass kernel (7x7 truncation).

Math: sigma = modulator[h,w] in (0,1); u = 1/(2 sigma^2 + 1e-8);
q = exp(-u) <= exp(-0.5).  Weight of tap (j,t) is q^(j^2+t^2).
Since q <= 0.6065, taps with |j| or |t| >= 4 carry < 5.4e-4 of the
kernel mass; normalizing by the truncated sum s3 = 1 + 2(q+q^4+q^9)
makes the truncated kernel a proper weighted average, so rel err from
truncation is ~5e-4.  m = j^2+t^2 groups kept: {1,2,4,5,8,9,10} and
m=10's partner merged as q^10 C10 ~= q^9 * (0.45 C10) (worst ~0.22%).

out = [Xc + q C1 + q^2 C2 + q^4 C4 + q^5 C5 + q^8 C8
        + q^9 (C9 + 0.45 C10)] / s3^2

Layout per core (8-way H-shard, 64 rows each):
  partitions p = rq*32+cb (4 row-quarters x 32 col-blocks),
  per-partition spatial block 16x16, X with halo rows 3 / cols 4
  -> X tile [128, 3ch, 22, 24] bf16.  A second copy XO shifted by one
  column keeps every column-pair add 4B-aligned for the DVE 2x bf16
  mode.  Weight maps are per-pixel [128,16,16], broadcast over the
  channel axis with stride-0 APs.
"""

import os
import numpy as np

H = W = 512
C = 3
NC = 8
RS = H // NC        # 64 rows per core
RQ = 4              # row-quarters per core
CB = 32             # col blocks
TR = 16             # block rows
TC = 16             # block cols
RHL = 3             # row halo
CHL = 4             # col halo (even => aligned bf16 slices)
XR = TR + 2 * RHL   # 22
XC = TC + 2 * CHL   # 24
P = 128

_NC_CACHE = {}


def _build_nc():
    if "nc" in _NC_CACHE:
        return _NC_CACHE["nc"]
    import concourse.bass as bass  # noqa: F401
    from concourse import bacc
    import concourse.mybir as mybir
    from concourse.tile import TileContext
    from concourse.ap import AP as BassAP

    f32 = mybir.dt.float32
    bf16 = mybir.dt.bfloat16
    AF = mybir.ActivationFunctionType
    ALU = mybir.AluOpType

    nc = bacc.Bacc()
    x = nc.dram_tensor("x", [P, C, XR, XC], bf16, kind="ExternalInput")
    xo = nc.dram_tensor("xo", [P, C, XR, XC], bf16, kind="ExternalInput")
    xn = nc.dram_tensor("xn", [P, C, XR, TC], bf16, kind="ExternalInput")
    md = nc.dram_tensor("md", [P, TR, TC], f32, kind="ExternalInput")
    out = nc.dram_tensor("out", [P, C, TR, TC], f32, kind="ExternalOutput")

    use_gp = os.environ.get("LGB2_GP", "0") == "1"
    nrep = int(os.environ.get("LGB2_REPEAT", "1"))

    with TileContext(nc) as tc:
        with tc.tile_pool(name="main", bufs=1) as pool:
            X = pool.tile([P, C, XR, XC], bf16, tag="X")
            XO = pool.tile([P, C, XR, XC], bf16, tag="XO")
            # XN: center cols only (no col halo) -> rows are contiguous, so
            # slot-strided multi-row-pair APs collapse to 3 free dims
            XN = pool.tile([P, C, XR, TC], bf16, tag="XN")
            MD = pool.tile([P, TR, TC], f32, tag="MD")
            nc.sync.dma_start(out=MD[:], in_=md[:])
            nc.sync.dma_start(out=X[:], in_=x[:])
            nc.sync.dma_start(out=XO[:], in_=xo[:])
            nc.sync.dma_start(out=XN[:], in_=xn[:])
            EPS = pool.tile([P, 1], f32, tag="EPS")
            nc.vector.memset(EPS[:], 1e-4)

            # all tiles allocated up-front so the compute body can sit
            # inside a hardware loop (repeat-timing mode)
            V = pool.tile([P, TR, TC], f32, tag="V")
            U = pool.tile([P, TR, TC], f32, tag="U")
            # slots: q1,q4,q9,q2,q5,q10,q8
            QAB = pool.tile([P, 7, TR, TC], bf16, tag="QAB")
            SS = pool.tile([P, TR, TC], f32, tag="SS")
            RN = pool.tile([P, TR, TC], f32, tag="RN")
            NRM = pool.tile([P, TR, TC], f32, tag="NRM")
            A = pool.tile([P, 3, C, XR, TC], bf16, tag="A")
            # CC slots: C1, C4, C9, C2, C5, C10, C8 | scratch: C5b, C10b
            CC = pool.tile([P, 9, C, TR, TC], bf16, tag="CC")
            # PR slots: P1, P4, P9, P2, P5, P10, P8
            PR = pool.tile([P, 7, C, TR, TC], bf16, tag="PR")
            L1 = pool.tile([P, 3, C, TR, TC], bf16, tag="L1")
            W1 = pool.tile([P, C, TR, TC], bf16, tag="W1")
            W2 = pool.tile([P, C, TR, TC], bf16, tag="W2")
            # F1 sums ~42% of the output mass -> bf16 rounding here is
            # <=0.17% worst-case; only F2 (adds the dominant center term)
            # and OUT stay fp32
            F1 = pool.tile([P, C, TR, TC], bf16, tag="F1")
            F2 = pool.tile([P, C, TR, TC], f32, tag="F2")
            OUTT = pool.tile([P, C, TR, TC], f32, tag="OUTT")

            def body():
                # ACT head: V = 2*sigma^2 + eps starts immediately
                nc.scalar.activation(V[:], MD[:], AF.Square,
                                     scale=float(np.sqrt(2.0)), bias=EPS[:])

                # ---- A_t: column pair sums (t-slot, ch, rows incl halo);
                # independent of the weight chain -> DVE is busy while ACT
                # squares and the u-chain result is awaited ----
                # A1 needs all 22 rows (j=3 shifts); A2 only rows 1..20,
                # A3 only rows 2..19 — skip halo rows nobody reads
                nc.vector.tensor_tensor(
                    A[:, 0], XO[:, :, :, 2:2 + TC], XO[:, :, :, 4:4 + TC],
                    ALU.add)
                nc.vector.tensor_tensor(
                    A[:, 1, :, 1:21], X[:, :, 1:21, 2:2 + TC],
                    X[:, :, 1:21, 6:6 + TC], ALU.add)
                nc.vector.tensor_tensor(
                    A[:, 2, :, 2:20], XO[:, :, 2:20, 0:TC],
                    XO[:, :, 2:20, 6:6 + TC], ALU.add)

                # ---- per-pixel u = 1/(2 sigma^2 + eps); eps folded into the
                # Square's bias ((r2*sig + 1e-4)^2 = 2 sig^2 + 1e-8 + tiny
                # cross term ~2.8e-4*sig, a ~3e-4 relative wobble on u) ----
                nc.vector.reciprocal_approx_fast(U[:], V[:])

                # ---- weight maps on ACT (overlap the DVE pair-sum stage) ----
                for i, m in enumerate((1, 4, 9, 2, 5, 10, 8)):
                    nc.scalar.activation(QAB[:, i], U[:], AF.Exp,
                                         scale=float(-m))

                # ---- X row-pairs (j=1,2,3) in ONE op: the slot axis walks
                # the row offset (stride -TC / +TC elements of the halo-free
                # XN copy, so dims collapse to 3) -> CC[0:3]; then += A
                # center rows -> CC slots (C1, C4, C9) ----
                def xslot(j0, slot_stride):
                    b = XN[:, None, :, j0:j0 + TR, :] \
                        .broadcast_to([P, 3, C, TR, TC])
                    ap2 = list(b.ap)
                    ap2[1] = [slot_stride * TC, 3]
                    return BassAP(b.tensor, b.offset, ap2)

                if os.environ.get("LGB2_XMERGE", "1") == "1":
                    nc.vector.tensor_tensor(
                        CC[:, 0:3], xslot(RHL - 1, -1), xslot(RHL + 1, 1),
                        ALU.add)
                else:
                    for i, j in enumerate((1, 2, 3)):
                        nc.vector.tensor_tensor(
                            CC[:, i],
                            XN[:, :, RHL - j:RHL - j + TR, :],
                            XN[:, :, RHL + j:RHL + j + TR, :],
                            ALU.add)
                nc.vector.tensor_tensor(
                    CC[:, 0:3], CC[:, 0:3], A[:, :, :, RHL:RHL + TR, :],
                    ALU.add)

                # ---- normalization 1/s^2 = exp(-2 ln s), s = 1+2(q1+q4+q9):
                # the recip moves to ACT (ln+exp share one table set) ----
                nc.vector.tensor_tensor(SS[:], QAB[:, 0], QAB[:, 1], ALU.add)
                nc.vector.tensor_tensor(SS[:], SS[:], QAB[:, 2], ALU.add)
                nc.scalar.activation(RN[:], SS[:], AF.Copy, bias=1.0,
                                     scale=2.0)
                nc.scalar.activation(RN[:], RN[:], AF.Ln)
                nc.scalar.activation(NRM[:], RN[:], AF.Exp, scale=-2.0)

                def arows(sl, j):
                    return (A[:, sl, :, RHL - j:RHL - j + TR, :],
                            A[:, sl, :, RHL + j:RHL + j + TR, :])

                # ---- A row-pairs ----
                lo, hi = arows(slice(0, 3), 1)
                nc.vector.tensor_tensor(CC[:, 3:6], lo, hi, ALU.add)
                # -> CC (.., C2, C5a, C10a)
                # j=2 pairs (C5b, C8) written slot-REVERSED so C8 lands at
                # CC[6] (joins the 7-slot product) and C5b at CC[7] (scratch)
                lo, hi = arows(slice(0, 2), 2)

                def rev2(apv):
                    ap2 = list(apv.ap)
                    sl = ap2[1]
                    assert sl[1] == 2
                    off = apv.offset + sl[0]
                    return BassAP(apv.tensor, off, [ap2[0], [-sl[0], 2]]
                                  + ap2[2:])

                nc.vector.tensor_tensor(rev2(CC[:, 6:8]), lo, hi, ALU.add)
                lo, hi = arows(0, 3)
                nc.vector.tensor_tensor(CC[:, 8], lo, hi, ALU.add)   # C10b
                # C5 += C5b ; C10 += C10b   (contiguous slot pair, one op)
                nc.vector.tensor_tensor(CC[:, 4:6], CC[:, 4:6], CC[:, 7:9],
                                        ALU.add)

                # ---- products ----
                def bc3(q):  # [P,TR,TC] -> [P,C,TR,TC] stride-0 channel bc
                    return q[:, None, :, :].broadcast_to([P, C, TR, TC])

                nc.vector.tensor_tensor(
                    PR[:],
                    QAB[:, :, None, :, :].broadcast_to([P, 7, C, TR, TC]),
                    CC[:, 0:7], ALU.mult)  # (P1, P4, P9, P2, P5, P10, P8)

                # ---- reduction tree (small terms in bf16, tail in fp32) ----
                nc.vector.tensor_tensor(L1[:], PR[:, 1:4], PR[:, 4:7],
                                        ALU.add)   # (P4+P2, P9+P5, P10+P8)
                nc.vector.tensor_tensor(W1[:], L1[:, 1], L1[:, 2], ALU.add)
                nc.vector.tensor_tensor(W2[:], W1[:], L1[:, 0], ALU.add)
                nc.vector.tensor_tensor(F1[:], W2[:], PR[:, 0], ALU.add)
                nc.vector.tensor_tensor(
                    F2[:], F1[:], XN[:, :, RHL:RHL + TR, :], ALU.add)
                nc.vector.tensor_tensor(OUTT[:], F2[:], bc3(NRM), ALU.mult)

            if nrep == 1:
                body()
            else:
                # 8x unrolled hw loop: the per-iteration For_i machinery
                # (~1.4 us) amortizes over 8 serial bodies in timing mode
                UN = 8
                assert nrep % UN == 0, nrep
                with tc.For_i(0, nrep // UN, 1):
                    for _ in range(UN):
                        body()
            nc.sync.dma_start(out=out[:], in_=OUTT[:])

    nc.compile()
    _NC_CACHE["nc"] = nc
    return nc


def _stage_inputs(img, modulator):
    import ml_dtypes
    x = np.ascontiguousarray(np.asarray(img, dtype=np.float32))[0]  # (3,H,W)
    mod = np.ascontiguousarray(np.asarray(modulator, dtype=np.float32))
    xpad = np.pad(x, ((0, 0), (RHL, RHL), (CHL, CHL + 1)), mode="edge")
    # (3, 518, 521)
    idx_r = (np.arange(RQ) * TR)[:, None] + np.arange(XR)[None, :]  # (4,22)
    idx_c = (np.arange(CB) * TC)[:, None] + np.arange(XC)[None, :]  # (32,24)
    idx_cn = (np.arange(CB) * TC)[:, None] + CHL + np.arange(TC)[None, :]
    mir = (np.arange(RQ) * TR)[:, None] + np.arange(TR)[None, :]
    mic = (np.arange(CB) * TC)[:, None] + np.arange(TC)[None, :]
    in_maps = []
    for core in range(NC):
        sub = xpad[:, core * RS:core * RS + RS + 2 * RHL, :]  # (3,70,521)
        # (3, 4, 32, 22, 24) -> (128, 3, 22, 24)
        blk = sub[:, idx_r[:, None, :, None], idx_c[None, :, None, :]]
        xt = np.ascontiguousarray(
            blk.transpose(1, 2, 0, 3, 4).reshape(P, C, XR, XC)
        ).astype(ml_dtypes.bfloat16)
        blk_o = sub[:, idx_r[:, None, :, None], idx_c[None, :, None, :] + 1]
        xot = np.ascontiguousarray(
            blk_o.transpose(1, 2, 0, 3, 4).reshape(P, C, XR, XC)
        ).astype(ml_dtypes.bfloat16)
        blk_n = sub[:, idx_r[:, None, :, None], idx_cn[None, :, None, :]]
        xnt = np.ascontiguousarray(
            blk_n.transpose(1, 2, 0, 3, 4).reshape(P, C, XR, TC)
        ).astype(ml_dtypes.bfloat16)
        msub = mod[core * RS:core * RS + RS, :]  # (64, 512)
        mdt = np.ascontiguousarray(
            msub[mir[:, None, :, None], mic[None, :, None, :]]
            .reshape(P, TR, TC))
        in_maps.append({"x": xt, "xo": xot, "xn": xnt, "md": mdt})
    return in_maps


def kernel(img, modulator):
    from concourse.bass_utils import run_bass_kernel_spmd

    nc = _build_nc()
    in_maps = _stage_inputs(img, modulator)
    res = run_bass_kernel_spmd(nc, in_maps, list(range(NC))).results
    # per-core out [128, 3, 16, 16] -> (3, 64, 512)
    parts = []
    for i in range(NC):
        o = np.asarray(res[i]["out"]).reshape(RQ, CB, C, TR, TC)
        parts.append(o.transpose(2, 0, 3, 1, 4).reshape(C, RS, W))
    out = np.concatenate(parts, axis=1)
    return np.ascontiguousarray(out[None], dtype=np.float32)
